# revision 44
# baseline (speedup 1.0000x reference)
"""Trainium2 Bass kernel for a single nGPT-style attention head.

Computation (see reference): fused QKV projection, RoPE over the full head
dim, L2-normalize q/k scaled by sqk, causal SDPA with scale sqrt(d_model).

Sharding: data-parallel over batch - 8 batch elements, one per NeuronCore.

Per-core layout: everything transposed, [d, t] with head/feature dim on
SBUF partitions. Software-pipelined per-block structure (TB=512 token
blocks): block j runs QKV matmuls, norms, RoPE, while the ATTENTION of
query block j-1 is interleaved strip-by-strip between the QKV matmul
groups so the scores-exp latency hides behind projection matmuls.

Engine assignment (keeps the scalar engine's activation table resident on
the exp-containing set for the whole kernel - a single ACT_TABLE_LOAD):
  PE:  QKV matmuls, rotate-half (permutation matmul), v transpose
       (identity matmul), transposed norm reductions (sq^T @ ones ->
       per-token partition layout), scores, attn@v, denominator (ones)
       accumulation.
  ACT: scores exp only, with the 1/||k|| fold done via the per-partition
       scale AP operand.
  DVE: psum->sbuf copies, squares, Quake-style rsqrt (bitcast + shift
       seed + 2 Newton steps), RoPE elementwise chain, causal triangle
       mask, reciprocal_approx_fast, final out normalize.
  GpSimd: inv-norm partition broadcasts only.
"""

import numpy as np
import ml_dtypes

import concourse.bass as bass
import concourse.tile as tile
from concourse import bacc, mybir
from concourse.bass import ts, ds
from concourse.bass_utils import run_bass_kernel_spmd

# Surface compile-hook exceptions (the PJRT bridge swallows tracebacks).
try:
    import traceback
    import libneuronxla as _lnx

    if not getattr(_lnx, "_err_wrapped", False):
        _orig_cc = _lnx.neuronx_cc

        def _cc_wrapper(*a, **kw):
            try:
                return _orig_cc(*a, **kw)
            except BaseException:
                traceback.print_exc()
                raise

        _lnx.neuronx_cc = _cc_wrapper
        _lnx._err_wrapped = True
except Exception:
    pass

AFT = mybir.ActivationFunctionType
ALU = mybir.AluOpType
F32 = mybir.dt.float32
BF16 = mybir.dt.bfloat16
I32 = mybir.dt.int32

B, T_FULL, C, D = 8, 2048, 1024, 128
ROPE_BASE = 10000.0
P = 128
TB = 512            # token block (tq block width = one PSUM bank of f32)
NCO = C // P        # contraction chunks for the QKV projection
H = P // 2


def build_nc(T=T_FULL, num_devices=8):
    from contextlib import ExitStack
    NTB = T // TB
    NKT = T // P
    nc = bacc.Bacc("TRN2", target_bir_lowering=False, debug=False,
                   num_devices=num_devices)

    xT = nc.dram_tensor("xT", [P, NTB, NCO, TB], BF16,
                        kind="ExternalInput").ap()
    WT = nc.dram_tensor("WT", [3, P, NCO, D], BF16,
                        kind="ExternalInput").ap()
    cosF = nc.dram_tensor("cosF", [P, T], BF16, kind="ExternalInput").ap()
    sinF = nc.dram_tensor("sinF", [P, T], BF16, kind="ExternalInput").ap()
    # packed small constants: [idn | smat | tri | onb | sqk232] (bf16)
    cpk = nc.dram_tensor("cpk", [P, 3 * P + 2], BF16,
                         kind="ExternalInput").ap()
    outT = nc.dram_tensor("outT", [D, T], F32, kind="ExternalOutput").ap()

    with tile.TileContext(nc) as tc:
        with ExitStack() as ctx:
            const = ctx.enter_context(tc.tile_pool(name="const", bufs=1))
            work = ctx.enter_context(tc.tile_pool(name="work", bufs=2))
            xpool = ctx.enter_context(tc.tile_pool(name="xpool", bufs=NTB))
            expool = ctx.enter_context(tc.tile_pool(name="expool", bufs=12))
            ps_big = ctx.enter_context(
                tc.tile_pool(name="ps_big", bufs=2, space="PSUM"))
            ps_sc = ctx.enter_context(
                tc.tile_pool(name="ps_sc", bufs=3, space="PSUM"))
            ps_o = ctx.enter_context(
                tc.tile_pool(name="ps_o", bufs=1, space="PSUM"))
            ps_d = ctx.enter_context(
                tc.tile_pool(name="ps_d", bufs=1, space="PSUM"))
            ps_sm = ctx.enter_context(
                tc.tile_pool(name="ps_sm", bufs=1, space="PSUM"))

            # critical-path DMAs on the SP queue: packed consts, q-weights,
            # x block 0; everything bulky rides the Activation DMA queue.
            cpk_sb = const.tile([P, 3 * P + 2], BF16)
            nc.sync.dma_start(cpk_sb, cpk)

            # engine warm-up: trigger each engine's library load and the
            # scalar engine's exp table load immediately, and give the PE a
            # few dummy matmuls so its clock ramps while the inputs stream.
            tin1 = work.tile([P, 8], F32, tag="tin1")
            nc.vector.memset(tin1, 1.0)
            tin2 = work.tile([P, 8], F32, tag="tin2")
            nc.gpsimd.memset(tin2, 1.0)
            tin3 = work.tile([P, 8], F32, tag="tin3")
            nc.scalar.activation(tin3, tin1, AFT.Exp)
            idn_sb = cpk_sb[:, 0:P]
            smat_sb = cpk_sb[:, P:2 * P]
            tri_sb = cpk_sb[:, 2 * P:3 * P]
            onb_sb = cpk_sb[:, 3 * P:3 * P + 1]
            sqk_sb = cpk_sb[:, 3 * P + 1:3 * P + 2]
            wt = const.tile([P, 3, NCO, D], BF16)
            xts = []
            xt0 = xpool.tile([P, NCO, TB], BF16, tag="xt", name="xt0")
            nc.sync.dma_start(wt[:, 0], WT[0])
            nc.sync.dma_start(xt0, xT[:, 0])
            nc.sync.dma_start(wt[:, 1], WT[1])
            nc.sync.dma_start(wt[:, 2], WT[2])
            xts.append(xt0)
            for w in range(6):
                wsc = ps_sc.tile([P, TB], F32, tag="sc", name=f"warm{w}")
                nc.tensor.matmul(wsc[:, 0:3 * P + 2], idn_sb, cpk_sb,
                                 start=True, stop=True)
            cos_sb = const.tile([P, T], BF16)
            nc.scalar.dma_start(cos_sb, cosF)
            sin_sb = const.tile([P, T], BF16)
            nc.scalar.dma_start(sin_sb, sinF)
            for j in range(1, NTB):
                xt = xpool.tile([P, NCO, TB], BF16, tag="xt", name=f"xt{j}")
                nc.scalar.dma_start(xt, xT[:, j])
                xts.append(xt)

            qk = const.tile([P, 2 * T], BF16)    # roped q^T | roped k^T
            vt = const.tile([P, NKT, P], BF16)   # v tiles [tk, e]

            # ---- pending attention work queue ----
            pend = []
            att_state = {}

            def drain(n):
                for _ in range(min(n, len(pend))):
                    pend.pop(0)()

            def make_att(J, part):
                """Queue attention strips for query block J.

                part 0: strips 0..4J-1 (need only PRIOR blocks' k/v) -
                queued right after rope J so they drain during block J's
                own v-projection. part 1: the 4 diagonal strips (need
                block J's k/v/invk) plus the AV tail and finish."""
                q_blk = qk[:, ts(J, TB)]
                nstr = 4 * (J + 1)
                if part == 0:
                    st = att_state[J] = {
                        "po": ps_o.tile([P, TB], F32, tag="po",
                                        name=f"po{J}"),
                        "ae": work.tile([P, TB], BF16, tag="ae",
                                        name=f"ae{J}"),
                        "ao": work.tile([P, TB], BF16, tag="ao",
                                        name=f"ao{J}"),
                        "exs": {},
                    }
                else:
                    st = att_state[J]
                po, ae, ao, exs = st["po"], st["ae"], st["ao"], st["exs"]

                def emit_scores(i):
                    dr = i - 4 * J
                    off = P * dr if dr >= 0 else 0
                    w = TB - off
                    sc = ps_sc.tile([P, TB], F32, tag="sc",
                                    name=f"sc{J}i{i}")
                    nc.tensor.matmul(sc[:, ds(off, w)],
                                     qk[:, ds(T + P * i, P)],
                                     q_blk[:, ds(off, w)],
                                     start=True, stop=True)
                    ex = expool.tile([P, TB], BF16, tag="ex",
                                     name=f"ex{J}i{i}")
                    nc.scalar.activation(ex[:, ds(off, w)],
                                         sc[:, ds(off, w)], AFT.Exp)
                    if dr >= 0:
                        nc.vector.tensor_mul(ex[:, ds(off, P)],
                                             ex[:, ds(off, P)], tri_sb)
                    exs[i] = (ex, off)

                def emit_av(i):
                    ex, off = exs.pop(i)
                    w = TB - off
                    nc.tensor.matmul(po[:, ds(off, w)], vt[:, i],
                                     ex[:, ds(off, w)],
                                     start=(i == 0), stop=(i == nstr - 1))
                    # denominator partial sums in bf16: even strips on DVE,
                    # odd strips on GpSimd; the deep ex ring lets these lag
                    # without gating the exp pipeline
                    eng, acc = (nc.vector, ae) if i % 2 == 0 else \
                        (nc.gpsimd, ao)
                    if i < 2:
                        if off > 0:
                            eng.memset(acc[:, ds(0, off)], 0.0)
                        eng.tensor_copy(acc[:, ds(off, w)],
                                        ex[:, ds(off, w)])
                    else:
                        eng.tensor_add(acc[:, ds(off, w)],
                                       acc[:, ds(off, w)],
                                       ex[:, ds(off, w)])

                def fin():
                    with nc.named_scope(f"fin{J}"):
                        red = ps_d.tile([1, TB], F32, tag="pd",
                                        name=f"red{J}")
                        nc.tensor.matmul(red, onb_sb, ae,
                                         start=True, stop=False)
                        nc.tensor.matmul(red, onb_sb, ao,
                                         start=False, stop=True)
                        invd = work.tile([1, TB], F32, tag="invd")
                        nc.vector.reciprocal_approx_fast(out=invd, in_=red)
                        bcd = work.tile([P, TB], F32, tag="bcd")
                        nc.gpsimd.partition_broadcast(bcd, invd)
                        ob = work.tile([P, TB], F32, tag="ob")
                        nc.vector.tensor_mul(ob, po, bcd)
                        nc.sync.dma_start(outT[:, ts(J, TB)], ob)

                def strip(i, J=J):
                    def run():
                        with nc.named_scope(f"att{J}s{i}"):
                            emit_scores(i)
                            if i >= 2:
                                emit_av(i - 2)
                    return run

                def last():
                    with nc.named_scope(f"att{J}tail"):
                        emit_av(nstr - 2)
                        emit_av(nstr - 1)
                        fin()

                if part == 0:
                    for i in range(nstr - 4):
                        pend.append(strip(i))
                else:
                    for i in range(nstr - 4, nstr):
                        pend.append(strip(i))
                    pend.append(last)

            def make_vblk(j, xt):
                """Queue the v projection + v transpose of block j."""
                def vmm():
                    with nc.named_scope(f"vblk{j}"):
                        st = att_state[j]
                        ps = st["vps"] = ps_big.tile(
                            [P, TB], F32, tag="big", name=f"qkv{j}g2")
                        for co in range(NCO):
                            nc.tensor.matmul(
                                ps, wt[:, 2, co], xt[:, co],
                                start=(co == 0), stop=(co == NCO - 1))

                def vtr():
                    with nc.named_scope(f"vtr{j}"):
                        vst = work.tile([P, TB], BF16, tag="vst")
                        nc.scalar.activation(vst, att_state[j]["vps"],
                                             AFT.Copy)
                        for c in range(4):
                            tp = ps_sm.tile([P, P], BF16, tag="small",
                                            name=f"vtp{j}c{c}")
                            nc.tensor.transpose(tp, vst[:, ts(c, P)],
                                                idn_sb)
                            nc.scalar.activation(vt[:, 4 * j + c], tp,
                                                 AFT.Copy)

                pend.append(vmm)
                pend.append(vtr)

            for j in range(NTB):
                xt = xts[j]
                # ------------- q/k projection + squares + norms -----------
                with nc.named_scope(f"qkv{j}"):
                    sq = work.tile([P, 2, TB], BF16, tag="sq")
                    for g in range(2):
                        ps = ps_big.tile([P, TB], F32, tag="big",
                                         name=f"qkv{j}g{g}")
                        for co in range(NCO):
                            nc.tensor.matmul(
                                ps, wt[:, g, co], xt[:, co],
                                start=(co == 0), stop=(co == NCO - 1))
                            if co in (2, 5):
                                drain(1)
                        dst = qk[:, ds(g * T + j * TB, TB)]
                        nc.scalar.activation(dst, ps, AFT.Copy)
                        nc.vector.tensor_mul(sq[:, g], dst, dst)
                        drain(1)

                    # transposed norm reductions: nrm[t_local, g*4+c]
                    nrm = ps_sm.tile([P, 8], F32, tag="small",
                                     name=f"nrm{j}")
                    for g in range(2):
                        for c in range(4):
                            nc.tensor.matmul(
                                nrm[:, ds(g * 4 + c, 1)],
                                sq[:, g, ts(c, P)], onb_sb,
                                start=True, stop=True)
                    nrs = work.tile([P, 8], F32, tag="nrs")
                    nc.scalar.activation(nrs, nrm, AFT.Copy)
                    drain(1)

                # ---- rsqrt of norms (Quake seed + 1 Newton step) on DVE ---
                with nc.named_scope(f"nrm{j}"):
                    hg = work.tile([P, 8], I32, tag="hg")
                    nc.vector.tensor_scalar(
                        out=hg, in0=nrs.bitcast(I32), scalar1=1,
                        scalar2=None, op0=ALU.logical_shift_right)
                    nc.vector.tensor_scalar(
                        out=hg, in0=hg, scalar1=-1.0,
                        scalar2=float(0x5F3759DF), op0=ALU.mult, op1=ALU.add)
                    y = hg.bitcast(F32)
                    a = work.tile([P, 8], F32, tag="a")
                    nc.vector.tensor_mul(a, y, y)
                    nc.vector.tensor_mul(a, a, nrs)
                    nc.vector.tensor_scalar(out=a, in0=a, scalar1=-0.5,
                                            scalar2=1.5, op0=ALU.mult,
                                            op1=ALU.add)
                    yb = work.tile([P, 8], BF16, tag="yb")
                    nc.vector.tensor_mul(yb, y, a)
                    drain(1)

                    # 1/||q||, 1/||k||: transpose each [128,1] column to a
                    # partition-0 row of a [1,TB] staging tile, broadcast
                    # across partitions on GpSimd.
                    iqt = work.tile([1, TB], F32, tag="iqt")
                    ikt = work.tile([1, TB], F32, tag="ikt")
                    for c in range(8):
                        tq = ps_sm.tile([1, P], BF16, tag="small",
                                        name=f"tq{j}c{c}")
                        nc.tensor.transpose(tq, yb[:, ds(c, 1)], idn_sb)
                        dstn = iqt if c < 4 else ikt
                        nc.vector.tensor_copy(dstn[:, ts(c % 4, P)], tq)
                    bcq = work.tile([P, TB], F32, tag="bcq")
                    nc.gpsimd.partition_broadcast(bcq, iqt)
                    bck = work.tile([P, TB], F32, tag="bck")
                    nc.gpsimd.partition_broadcast(bck, ikt)
                    drain(1)

                # ------------------------- RoPE ---------------------------
                with nc.named_scope(f"rope{j}"):
                    ch_t = ds(j * TB, TB)
                    for part in range(2):  # 0 = q, 1 = k
                        chq = ds(part * T + j * TB, TB)
                        rot = ps_big.tile([P, TB], F32, tag="big",
                                          name=f"rot{j}p{part}")
                        nc.tensor.matmul(rot, smat_sb, qk[:, chq],
                                         start=True, stop=True)
                        t2 = work.tile([P, TB], BF16, tag="t2")
                        nc.vector.tensor_mul(t2, rot, sin_sb[:, ch_t])
                        t1 = work.tile([P, TB], BF16, tag="t1")
                        nc.vector.tensor_mul(t1, qk[:, chq], cos_sb[:, ch_t])
                        nc.vector.tensor_add(t1, t1, t2)
                        if part == 0:
                            nc.vector.scalar_tensor_tensor(
                                out=qk[:, chq], in0=t1, scalar=sqk_sb,
                                in1=bcq, op0=ALU.mult, op1=ALU.mult)
                        else:
                            nc.vector.tensor_mul(qk[:, chq], t1, bck)
                        drain(1)

                # queue: attention state + off-diagonal strips, the v
                # block, then the diagonal strips + finish. All of it
                # drains behind the later block fronts.
                make_att(j, 0)
                make_vblk(j, xt)
                make_att(j, 1)
            drain(len(pend))

    nc.compile()
    return nc


def _host_tables(T):
    d = D
    inv_freq = 1.0 / (ROPE_BASE ** (np.arange(0, d, 2, dtype=np.float64) / d))
    t = np.arange(T, dtype=np.float64)
    freqs = np.outer(inv_freq, t)                 # [d/2, T]
    emb = np.concatenate([freqs, freqs], axis=0)  # [d, T]
    cos1 = np.cos(emb)
    sin1 = np.sin(emb)
    # rotate_half sign folded into the sin table: the device rot is a plain
    # half-swap permutation; sin rows 0:d/2 carry the minus sign.
    sin1[: d // 2, :] *= -1.0
    cosF = cos1.astype(ml_dtypes.bfloat16)
    sinF = sin1.astype(ml_dtypes.bfloat16)
    a = np.arange(P)
    tri = (a[None, :] >= a[:, None]).astype(ml_dtypes.bfloat16)  # [tk, tq]
    idn = np.eye(P, dtype=ml_dtypes.bfloat16)
    smat = np.zeros((P, P), dtype=ml_dtypes.bfloat16)
    smat[np.arange(H) + H, np.arange(H)] = 1     # out[i<64]  = q[i+64]
    smat[np.arange(H), np.arange(H) + H] = 1     # out[i>=64] = q[i-64]
    return cosF, sinF, tri, idn, smat


TRACE = False
LAST_EXEC_NS = None
LAST_TRACE = None
LAST_INSTS = None


def kernel(x, W_qkv, sqk):
    global LAST_EXEC_NS, LAST_TRACE, LAST_INSTS
    T = x.shape[1]
    NTB = T // TB
    cosF, sinF, tri, idn, smat = _host_tables(T)
    # WT[g, p, co, d] = W_qkv[g*D + d, co*P + p]
    WT = np.ascontiguousarray(
        np.asarray(W_qkv).reshape(3, D, NCO, P).transpose(0, 3, 2, 1)
    ).astype(ml_dtypes.bfloat16)
    sqk232 = ((C ** 0.5) * np.asarray(sqk, np.float64) ** 2).reshape(D, 1)
    onb = np.ones((P, 1), np.float64)
    cpkh = np.concatenate(
        [idn.astype(np.float64), smat.astype(np.float64),
         tri.astype(np.float64), onb, sqk232],
        axis=1).astype(ml_dtypes.bfloat16)
    in_maps = []
    for b in range(B):
        xb = np.asarray(x[b]).T.astype(ml_dtypes.bfloat16)   # [C, T]
        xb = np.ascontiguousarray(
            xb.reshape(NCO, P, NTB, TB).transpose(1, 2, 0, 3))
        in_maps.append({
            "xT": xb,
            "WT": WT,
            "cosF": cosF,
            "sinF": sinF,
            "cpk": cpkh,
        })
    nc = build_nc(T=T, num_devices=B)
    res = run_bass_kernel_spmd(nc, in_maps, core_ids=list(range(B)),
                               trace=TRACE)
    LAST_EXEC_NS = res.exec_time_ns
    LAST_TRACE = (res.instructions_and_trace[1]
                  if res.instructions_and_trace else None)
    LAST_INSTS = (res.instructions_and_trace[0]
                  if res.instructions_and_trace else None)
    out = np.stack([r["outT"].T for r in res.results])  # [B, T, D]
    return np.ascontiguousarray(out).astype(np.float32)


# revision 45
# speedup vs baseline: 1.5718x; 1.5718x over previous
"""Trainium2 Bass kernel for a single nGPT-style attention head.

Computation (see reference): fused QKV projection, RoPE over the full head
dim, L2-normalize q/k scaled by sqk, causal SDPA with scale sqrt(d_model).

Sharding: data-parallel over batch - 8 batch elements, one per NeuronCore.

Per-core layout: everything transposed, [d, t] with head/feature dim on
SBUF partitions. Software-pipelined per-block structure (TB=512 token
blocks): block j runs QKV matmuls, norms, RoPE, while the ATTENTION of
query block j-1 is interleaved strip-by-strip between the QKV matmul
groups so the scores-exp latency hides behind projection matmuls.

Engine assignment (keeps the scalar engine's activation table resident on
the exp-containing set for the whole kernel - a single ACT_TABLE_LOAD):
  PE:  QKV matmuls, rotate-half (permutation matmul), v transpose
       (identity matmul), transposed norm reductions (sq^T @ ones ->
       per-token partition layout), scores, attn@v, denominator (ones)
       accumulation.
  ACT: scores exp only, with the 1/||k|| fold done via the per-partition
       scale AP operand.
  DVE: psum->sbuf copies, squares, Quake-style rsqrt (bitcast + shift
       seed + 2 Newton steps), RoPE elementwise chain, causal triangle
       mask, reciprocal_approx_fast, final out normalize.
  GpSimd: inv-norm partition broadcasts only.
"""

import numpy as np
import ml_dtypes

import concourse.bass as bass
import concourse.tile as tile
from concourse import bacc, mybir
from concourse.bass import ts, ds
from concourse.bass_utils import run_bass_kernel_spmd

# Surface compile-hook exceptions (the PJRT bridge swallows tracebacks).
try:
    import traceback
    import libneuronxla as _lnx

    if not getattr(_lnx, "_err_wrapped", False):
        _orig_cc = _lnx.neuronx_cc

        def _cc_wrapper(*a, **kw):
            try:
                return _orig_cc(*a, **kw)
            except BaseException:
                traceback.print_exc()
                raise

        _lnx.neuronx_cc = _cc_wrapper
        _lnx._err_wrapped = True
except Exception:
    pass

AFT = mybir.ActivationFunctionType
ALU = mybir.AluOpType
F32 = mybir.dt.float32
BF16 = mybir.dt.bfloat16
I32 = mybir.dt.int32

B, T_FULL, C, D = 8, 2048, 1024, 128
ROPE_BASE = 10000.0
P = 128
TB = 512            # token block (tq block width = one PSUM bank of f32)
NCO = C // P        # contraction chunks for the QKV projection
H = P // 2


def build_nc(T=T_FULL, num_devices=8):
    from contextlib import ExitStack
    NTB = T // TB
    NKT = T // P
    nc = bacc.Bacc("TRN2", target_bir_lowering=False, debug=False,
                   num_devices=num_devices)

    xT = nc.dram_tensor("xT", [P, NTB, NCO, TB], BF16,
                        kind="ExternalInput").ap()
    WT = nc.dram_tensor("WT", [3, P, NCO, D], BF16,
                        kind="ExternalInput").ap()
    cosF = nc.dram_tensor("cosF", [P, T], BF16, kind="ExternalInput").ap()
    sinF = nc.dram_tensor("sinF", [P, T], BF16, kind="ExternalInput").ap()
    # packed small constants: [idn | smat | tri | onb | sqk232] (bf16)
    cpk = nc.dram_tensor("cpk", [P, 3 * P + 2], BF16,
                         kind="ExternalInput").ap()
    outT = nc.dram_tensor("outT", [D, T], F32, kind="ExternalOutput").ap()

    with tile.TileContext(nc) as tc:
        with ExitStack() as ctx:
            const = ctx.enter_context(tc.tile_pool(name="const", bufs=1))
            work = ctx.enter_context(tc.tile_pool(name="work", bufs=2))
            xpool = ctx.enter_context(tc.tile_pool(name="xpool", bufs=NTB))
            expool = ctx.enter_context(tc.tile_pool(name="expool", bufs=12))
            ps_big = ctx.enter_context(
                tc.tile_pool(name="ps_big", bufs=2, space="PSUM"))
            ps_sc = ctx.enter_context(
                tc.tile_pool(name="ps_sc", bufs=3, space="PSUM"))
            ps_o = ctx.enter_context(
                tc.tile_pool(name="ps_o", bufs=1, space="PSUM"))
            ps_d = ctx.enter_context(
                tc.tile_pool(name="ps_d", bufs=1, space="PSUM"))
            ps_sm = ctx.enter_context(
                tc.tile_pool(name="ps_sm", bufs=1, space="PSUM"))

            # critical-path DMAs on the SP queue: packed consts, q-weights,
            # x block 0; everything bulky rides the Activation DMA queue.
            cpk_sb = const.tile([P, 3 * P + 2], BF16)
            nc.sync.dma_start(cpk_sb, cpk)

            # engine warm-up: trigger each engine's library load and the
            # scalar engine's exp table load immediately, and give the PE a
            # few dummy matmuls so its clock ramps while the inputs stream.
            tin1 = work.tile([P, 8], F32, tag="tin1")
            nc.vector.memset(tin1, 1.0)
            tin2 = work.tile([P, 8], F32, tag="tin2")
            nc.gpsimd.memset(tin2, 1.0)
            tin3 = work.tile([P, 8], F32, tag="tin3")
            nc.scalar.activation(tin3, tin1, AFT.Exp)
            idn_sb = cpk_sb[:, 0:P]
            smat_sb = cpk_sb[:, P:2 * P]
            tri_sb = cpk_sb[:, 2 * P:3 * P]
            onb_sb = cpk_sb[:, 3 * P:3 * P + 1]
            sqk_sb = cpk_sb[:, 3 * P + 1:3 * P + 2]
            wt = const.tile([P, 3, NCO, D], BF16)
            xts = []
            xt0 = xpool.tile([P, NCO, TB], BF16, tag="xt", name="xt0")
            nc.sync.dma_start(wt[:, 0], WT[0])
            nc.sync.dma_start(xt0, xT[:, 0])
            nc.sync.dma_start(wt[:, 1], WT[1])
            nc.sync.dma_start(wt[:, 2], WT[2])
            xts.append(xt0)
            for w in range(6):
                wsc = ps_sc.tile([P, TB], F32, tag="sc", name=f"warm{w}")
                nc.tensor.matmul(wsc[:, 0:3 * P + 2], idn_sb, cpk_sb,
                                 start=True, stop=True)
            cos_sb = const.tile([P, T], BF16)
            nc.scalar.dma_start(cos_sb, cosF)
            sin_sb = const.tile([P, T], BF16)
            nc.scalar.dma_start(sin_sb, sinF)
            for j in range(1, NTB):
                xt = xpool.tile([P, NCO, TB], BF16, tag="xt", name=f"xt{j}")
                nc.scalar.dma_start(xt, xT[:, j])
                xts.append(xt)

            qk = const.tile([P, 2 * T], BF16)    # roped q^T | roped k^T
            vt = const.tile([P, NKT, P], BF16)   # v tiles [tk, e]

            # ---- pending attention work queue ----
            pend = []
            att_state = {}

            def drain(n):
                for _ in range(min(n, len(pend))):
                    pend.pop(0)()

            def make_att(J, part):
                """Queue attention strips for query block J.

                part 0: strips 0..4J-1 (need only PRIOR blocks' k/v) -
                queued right after rope J so they drain during block J's
                own v-projection. part 1: the 4 diagonal strips (need
                block J's k/v/invk) plus the AV tail and finish."""
                q_blk = qk[:, ts(J, TB)]
                nstr = 4 * (J + 1)
                if part == 0:
                    st = att_state[J] = {
                        "po": ps_o.tile([P, TB], F32, tag="po",
                                        name=f"po{J}"),
                        "ae": work.tile([P, TB], BF16, tag="ae",
                                        name=f"ae{J}"),
                        "exs": {},
                    }
                else:
                    st = att_state[J]
                po, ae, exs = st["po"], st["ae"], st["exs"]

                def emit_scores(i):
                    dr = i - 4 * J
                    off = P * dr if dr >= 0 else 0
                    w = TB - off
                    sc = ps_sc.tile([P, TB], F32, tag="sc",
                                    name=f"sc{J}i{i}")
                    nc.tensor.matmul(sc[:, ds(off, w)],
                                     qk[:, ds(T + P * i, P)],
                                     q_blk[:, ds(off, w)],
                                     start=True, stop=True)
                    ex = expool.tile([P, TB], BF16, tag="ex",
                                     name=f"ex{J}i{i}")
                    nc.scalar.activation(ex[:, ds(off, w)],
                                         sc[:, ds(off, w)], AFT.Exp)
                    if dr >= 0:
                        nc.vector.tensor_mul(ex[:, ds(off, P)],
                                             ex[:, ds(off, P)], tri_sb)
                    exs[i] = (ex, off)

                def emit_av(i):
                    ex, off = exs.pop(i)
                    w = TB - off
                    nc.tensor.matmul(po[:, ds(off, w)], vt[:, i],
                                     ex[:, ds(off, w)],
                                     start=(i == 0), stop=(i == nstr - 1))
                    # denominator partial sums in bf16 on DVE; the deep
                    # ex ring lets these lag without gating the exps
                    if i == 0:
                        nc.vector.tensor_copy(ae, ex)
                    else:
                        nc.vector.tensor_add(ae[:, ds(off, w)],
                                             ae[:, ds(off, w)],
                                             ex[:, ds(off, w)])

                def fin():
                    with nc.named_scope(f"fin{J}"):
                        red = ps_d.tile([1, TB], F32, tag="pd",
                                        name=f"red{J}")
                        nc.tensor.matmul(red, onb_sb, ae,
                                         start=True, stop=True)
                        invd = work.tile([1, TB], F32, tag="invd")
                        nc.vector.reciprocal_approx_fast(out=invd, in_=red)
                        bcd = work.tile([P, TB], F32, tag="bcd")
                        nc.gpsimd.partition_broadcast(bcd, invd)
                        ob = work.tile([P, TB], F32, tag="ob")
                        nc.vector.tensor_mul(ob, po, bcd)
                        nc.sync.dma_start(outT[:, ts(J, TB)], ob)

                def strip(i, J=J):
                    def run():
                        with nc.named_scope(f"att{J}s{i}"):
                            emit_scores(i)
                            if i >= 2:
                                emit_av(i - 2)
                    return run

                def last():
                    with nc.named_scope(f"att{J}tail"):
                        emit_av(nstr - 2)
                        emit_av(nstr - 1)
                        fin()

                if part == 0:
                    for i in range(nstr - 4):
                        pend.append(strip(i))
                else:
                    for i in range(nstr - 4, nstr):
                        pend.append(strip(i))
                    pend.append(last)

            def make_vblk(j, xt):
                """Queue the v projection + v transpose of block j."""
                def vmm():
                    with nc.named_scope(f"vblk{j}"):
                        st = att_state[j]
                        ps = st["vps"] = ps_big.tile(
                            [P, TB], F32, tag="big", name=f"qkv{j}g2")
                        for co in range(NCO):
                            nc.tensor.matmul(
                                ps, wt[:, 2, co], xt[:, co],
                                start=(co == 0), stop=(co == NCO - 1))

                def vtr():
                    with nc.named_scope(f"vtr{j}"):
                        vst = work.tile([P, TB], BF16, tag="vst")
                        nc.scalar.activation(vst, att_state[j]["vps"],
                                             AFT.Copy)
                        for c in range(4):
                            tp = ps_sm.tile([P, P], BF16, tag="small",
                                            name=f"vtp{j}c{c}")
                            nc.tensor.transpose(tp, vst[:, ts(c, P)],
                                                idn_sb)
                            nc.vector.tensor_copy(vt[:, 4 * j + c], tp)

                pend.append(vmm)
                pend.append(vtr)

            for j in range(NTB):
                xt = xts[j]
                # ------------- q/k projection + squares + norms -----------
                with nc.named_scope(f"qkv{j}"):
                    sq = work.tile([P, 2, TB], BF16, tag="sq")
                    for g in range(2):
                        ps = ps_big.tile([P, TB], F32, tag="big",
                                         name=f"qkv{j}g{g}")
                        for co in range(NCO):
                            nc.tensor.matmul(
                                ps, wt[:, g, co], xt[:, co],
                                start=(co == 0), stop=(co == NCO - 1))
                            if co in (2, 5):
                                drain(1)
                        dst = qk[:, ds(g * T + j * TB, TB)]
                        nc.scalar.activation(dst, ps, AFT.Copy)
                        nc.vector.tensor_mul(sq[:, g], dst, dst)
                        drain(1)

                    # transposed norm reductions: nrm[t_local, g*4+c]
                    nrm = ps_sm.tile([P, 8], F32, tag="small",
                                     name=f"nrm{j}")
                    for g in range(2):
                        for c in range(4):
                            nc.tensor.matmul(
                                nrm[:, ds(g * 4 + c, 1)],
                                sq[:, g, ts(c, P)], onb_sb,
                                start=True, stop=True)
                    nrs = work.tile([P, 8], F32, tag="nrs")
                    nc.scalar.activation(nrs, nrm, AFT.Copy)
                    drain(1)

                # ---- rsqrt of norms (Quake seed + 1 Newton step) on DVE ---
                with nc.named_scope(f"nrm{j}"):
                    hg = work.tile([P, 8], I32, tag="hg")
                    nc.vector.tensor_scalar(
                        out=hg, in0=nrs.bitcast(I32), scalar1=1,
                        scalar2=None, op0=ALU.logical_shift_right)
                    nc.vector.tensor_scalar(
                        out=hg, in0=hg, scalar1=-1.0,
                        scalar2=float(0x5F3759DF), op0=ALU.mult, op1=ALU.add)
                    y = hg.bitcast(F32)
                    a = work.tile([P, 8], F32, tag="a")
                    nc.vector.tensor_mul(a, y, y)
                    nc.vector.tensor_mul(a, a, nrs)
                    nc.vector.tensor_scalar(out=a, in0=a, scalar1=-0.5,
                                            scalar2=1.5, op0=ALU.mult,
                                            op1=ALU.add)
                    yb = work.tile([P, 8], BF16, tag="yb")
                    nc.vector.tensor_mul(yb, y, a)
                    drain(1)

                    # 1/||q||, 1/||k||: transpose each [128,1] column to a
                    # partition-0 row of a [1,TB] staging tile, broadcast
                    # across partitions on GpSimd.
                    iqt = work.tile([1, TB], F32, tag="iqt")
                    ikt = work.tile([1, TB], F32, tag="ikt")
                    for c in range(8):
                        tq = ps_sm.tile([1, P], BF16, tag="small",
                                        name=f"tq{j}c{c}")
                        nc.tensor.transpose(tq, yb[:, ds(c, 1)], idn_sb)
                        dstn = iqt if c < 4 else ikt
                        nc.vector.tensor_copy(dstn[:, ts(c % 4, P)], tq)
                    bcq = work.tile([P, TB], F32, tag="bcq")
                    nc.gpsimd.partition_broadcast(bcq, iqt)
                    bck = work.tile([P, TB], F32, tag="bck")
                    nc.gpsimd.partition_broadcast(bck, ikt)
                    drain(1)

                # ------------------------- RoPE ---------------------------
                with nc.named_scope(f"rope{j}"):
                    ch_t = ds(j * TB, TB)
                    for part in range(2):  # 0 = q, 1 = k
                        chq = ds(part * T + j * TB, TB)
                        rot = ps_big.tile([P, TB], F32, tag="big",
                                          name=f"rot{j}p{part}")
                        nc.tensor.matmul(rot, smat_sb, qk[:, chq],
                                         start=True, stop=True)
                        t2 = work.tile([P, TB], BF16, tag="t2")
                        nc.vector.tensor_mul(t2, rot, sin_sb[:, ch_t])
                        t1 = work.tile([P, TB], BF16, tag="t1")
                        nc.vector.tensor_mul(t1, qk[:, chq], cos_sb[:, ch_t])
                        nc.vector.tensor_add(t1, t1, t2)
                        if part == 0:
                            nc.vector.scalar_tensor_tensor(
                                out=qk[:, chq], in0=t1, scalar=sqk_sb,
                                in1=bcq, op0=ALU.mult, op1=ALU.mult)
                        else:
                            nc.vector.tensor_mul(qk[:, chq], t1, bck)
                        drain(1)

                # queue: attention state + off-diagonal strips, the v
                # block, then the diagonal strips + finish. All of it
                # drains behind the later block fronts.
                make_att(j, 0)
                make_vblk(j, xt)
                make_att(j, 1)
            drain(len(pend))

    nc.compile()
    return nc


def _host_tables(T):
    d = D
    inv_freq = 1.0 / (ROPE_BASE ** (np.arange(0, d, 2, dtype=np.float64) / d))
    t = np.arange(T, dtype=np.float64)
    freqs = np.outer(inv_freq, t)                 # [d/2, T]
    emb = np.concatenate([freqs, freqs], axis=0)  # [d, T]
    cos1 = np.cos(emb)
    sin1 = np.sin(emb)
    # rotate_half sign folded into the sin table: the device rot is a plain
    # half-swap permutation; sin rows 0:d/2 carry the minus sign.
    sin1[: d // 2, :] *= -1.0
    cosF = cos1.astype(ml_dtypes.bfloat16)
    sinF = sin1.astype(ml_dtypes.bfloat16)
    a = np.arange(P)
    tri = (a[None, :] >= a[:, None]).astype(ml_dtypes.bfloat16)  # [tk, tq]
    idn = np.eye(P, dtype=ml_dtypes.bfloat16)
    smat = np.zeros((P, P), dtype=ml_dtypes.bfloat16)
    smat[np.arange(H) + H, np.arange(H)] = 1     # out[i<64]  = q[i+64]
    smat[np.arange(H), np.arange(H) + H] = 1     # out[i>=64] = q[i-64]
    return cosF, sinF, tri, idn, smat


TRACE = False
LAST_EXEC_NS = None
LAST_TRACE = None
LAST_INSTS = None


def kernel(x, W_qkv, sqk):
    global LAST_EXEC_NS, LAST_TRACE, LAST_INSTS
    T = x.shape[1]
    NTB = T // TB
    cosF, sinF, tri, idn, smat = _host_tables(T)
    # WT[g, p, co, d] = W_qkv[g*D + d, co*P + p]
    WT = np.ascontiguousarray(
        np.asarray(W_qkv).reshape(3, D, NCO, P).transpose(0, 3, 2, 1)
    ).astype(ml_dtypes.bfloat16)
    sqk232 = ((C ** 0.5) * np.asarray(sqk, np.float64) ** 2).reshape(D, 1)
    onb = np.ones((P, 1), np.float64)
    cpkh = np.concatenate(
        [idn.astype(np.float64), smat.astype(np.float64),
         tri.astype(np.float64), onb, sqk232],
        axis=1).astype(ml_dtypes.bfloat16)
    in_maps = []
    for b in range(B):
        xb = np.asarray(x[b]).T.astype(ml_dtypes.bfloat16)   # [C, T]
        xb = np.ascontiguousarray(
            xb.reshape(NCO, P, NTB, TB).transpose(1, 2, 0, 3))
        in_maps.append({
            "xT": xb,
            "WT": WT,
            "cosF": cosF,
            "sinF": sinF,
            "cpk": cpkh,
        })
    nc = build_nc(T=T, num_devices=B)
    res = run_bass_kernel_spmd(nc, in_maps, core_ids=list(range(B)),
                               trace=TRACE)
    LAST_EXEC_NS = res.exec_time_ns
    LAST_TRACE = (res.instructions_and_trace[1]
                  if res.instructions_and_trace else None)
    LAST_INSTS = (res.instructions_and_trace[0]
                  if res.instructions_and_trace else None)
    out = np.stack([r["outT"].T for r in res.results])  # [B, T, D]
    return np.ascontiguousarray(out).astype(np.float32)
